# revision 10
# baseline (speedup 1.0000x reference)
"""Trainium2 Bass kernel for the NeighborhoodAttention module (v2).

Data-parallel over B across 8 cores; table + weights replicated.

Math (per batch row b):
    center = E[ci[b]];  k[b,j] = (E @ (SCALE*Wk).T)[ni[b,j]]   (kall on host)
    q      = center @ Wq.T
    logits[j] = <k[b,j], q[b]> + log(clip(w[b,j]))             (logw on host)
    attn   = softmax(logits)            (no max-sub: logits bounded)
    gate   = sigmoid(center@Wg+bg) = (1+tanh((center@Wg+bg)/2))/2
             (the sigmoid 1/2 is folded into W2)
    ctx    = sum_j attn[j]*nbs[j];  cg = (1+tanh)*ctx
    x      = center @ (Wo1+I).T + cg @ (Wo2/2).T + bo
    out    = (x-mu)*rsqrt(var+eps)      (gamma/beta applied on host)

v2 design vs v1: the per-tile diagonal-stationary construction (DVE/ACT
scaled identity copies + gpsimd local_scatter) dominated HW time.  It is
replaced by a block-diagonal formulation:

  * neighbor embeddings are gathered in b-block order: gather position
    (within a 128-row tile) = 512*t + 128*m + 4*u + jj for batch row
    b = 32t+u and neighbor j = 4m+jj.  So PSUM slot s = 4t+m holds, at
    partition p = 4u+jj, the embedding of (b=32t+u, j=4m+jj).
  * the k-rows are gathered separately in j-major order (as v1) so
    logits/softmax stay per-partition ops.
  * attn [128b,16j] is transposed once on PE, replicated down partitions
    with 4 const matmuls (stat4), masked with one DVE multiply
    (maskB4) -> bigb[p, m*128+b] = attn[b, 4m+p%4] * (p//4 == b%32).
  * ctx for block t accumulates 4 matmuls (m=0..3):
      ctx[32t:32t+32, :] += bigb[:, m*128+32t : +32].T @ catg[:, 4t+m, :]
    using PE column tiles at partition offsets {0,32,64,96}.

Table row (1024B, f32-viewed [N, 256]):
    [emb bf16 512B | kall bf16 128B | pad 128B | emb fp8e4m3 256B]
  centers: transposed bf16 gather of cols 0:256(bf16);
  k: 256B gather of f32 cols 128:192; neighbors: 256B fp8 gather of
  f32 cols 192:256 (fp8 keeps the 256B-multiple gather constraint at
  half the bytes of bf16; attn stationaries stay bf16 so tiny attn
  weights do not flush).

All small inputs are packed into one [128, CB] f32 "blob" (bitcast
views carve out i16/bf16 regions) so each dispatch carries 2 input
handles instead of 12 (per-dispatch RPC overhead scales with handle
count under axon).
"""

import os
from contextlib import ExitStack

import numpy as np

import concourse.tile as tile
from concourse import bacc, mybir

D = 256
A = 64
K = 16
N = 20000
B = 32768
NCORES = 8
BC = B // NCORES          # 4096 rows per core
SCALE = A ** -0.5
EPS = 1e-5

CATF = 256                # f32 cols per table row (1024B)

# rsqrt(v) seed constants: y0 = (RC*r + RB)*(r + RA), r = 1/v, v in [0.55, 2.4]
RA = 0.402005013
RB = 0.852322024
RC = -0.137923577

f32 = mybir.dt.float32
bf16 = mybir.dt.bfloat16
fp8 = mybir.dt.float8e4
i16 = mybir.dt.int16

FN = mybir.ActivationFunctionType
OP = mybir.AluOpType

# blob layout in f32 columns
def _blob_layout(tiles):
    pairs = tiles // 2
    c = {}
    off = 0
    def put(name, cols):
        nonlocal off
        c[name] = (off, off + cols)
        off += cols
    put("nidx", tiles * 64)        # i16 [128, tiles*128]
    put("kidx", tiles * 64)        # i16 [128, tiles*128]
    put("cidx", pairs * 8)         # i16 [128, pairs*16]
    put("logw", tiles * 16)        # f32 [128, tiles*16]
    put("wq", 64)                  # bf16 [128, 128]
    put("wg", 256)                 # bf16 [128, 512]
    put("w1", 256)                 # bf16 [128, 512]
    put("w2", 256)                 # bf16 [128, 512]
    put("bgbo", 256)               # bf16 [128, 512] (row 0 used)
    put("identb", 64)              # bf16 [128, 128]
    put("stat4", 256)              # bf16 [128, 512] (rows 0:16 used)
    put("maskB4", 256)             # bf16 [128, 512]
    put("ones", 64)                # bf16 [128, 128] (row 0 used)
    return c, off


def build_program(tiles=BC // 128, reps=1):
    """Builds the per-core program.  `tiles` = number of 128-row tiles.
    `reps` replicates the whole tile loop (timing probes only)."""
    nc = bacc.Bacc("TRN2", target_bir_lowering=False, debug=False)
    pairs = tiles // 2
    assert tiles % 2 == 0
    lay, CB = _blob_layout(tiles)

    catt = nc.dram_tensor("catt", [N, CATF], f32, kind="ExternalInput")
    blob = nc.dram_tensor("blob", [128, CB], f32, kind="ExternalInput")
    out_d = nc.dram_tensor("out", [tiles * 128, D], bf16, kind="ExternalOutput")

    with tile.TileContext(nc) as tc, ExitStack() as ctx:
        const = ctx.enter_context(tc.tile_pool(name="const", bufs=1))
        blob_sb = const.tile([128, CB], f32)
        # stage first-pair indices first so gathers start early
        i0, _ = lay["nidx"]; k0, _ = lay["kidx"]; c0, _ = lay["cidx"]
        nc.sync.dma_start(blob_sb[:, i0:i0 + 128], blob.ap()[:, i0:i0 + 128])
        nc.sync.dma_start(blob_sb[:, k0:k0 + 128], blob.ap()[:, k0:k0 + 128])
        nc.sync.dma_start(blob_sb[:, c0:c0 + 8], blob.ap()[:, c0:c0 + 8])
        if i0 + 128 < k0:
            nc.sync.dma_start(blob_sb[:, i0 + 128:k0],
                              blob.ap()[:, i0 + 128:k0])
        if k0 + 128 < c0:
            nc.sync.dma_start(blob_sb[:, k0 + 128:c0],
                              blob.ap()[:, k0 + 128:c0])
        nc.sync.dma_start(blob_sb[:, c0 + 8:], blob.ap()[:, c0 + 8:])

        def view(name, dt=None):
            a, b = lay[name]
            v = blob_sb[:, a:b]
            return v.bitcast(dt) if dt is not None else v

        idxN = view("nidx", i16)           # [128, tiles*128]
        idxK = view("kidx", i16)
        idxC = view("cidx", i16)           # [128, pairs*16]
        logw = view("logw")                # [128, tiles*16] f32
        wq = view("wq", bf16)              # [128, 128]
        wg = view("wg", bf16)              # [128, 512]
        w1 = view("w1", bf16)
        w2 = view("w2", bf16)
        bgbo = view("bgbo", bf16)          # [128, 512]
        idb = view("identb", bf16)         # [128, 128]
        stat4 = view("stat4", bf16)        # [128, 512]
        maskB4 = view("maskB4", bf16)      # [128, 512]
        onesv = view("ones", bf16)         # [128, 128]
        bg_row = bgbo[0:1, 0:D]
        bo_row = bgbo[0:1, D:2 * D]
        ones1 = onesv[0:1, :]

        catg_p = ctx.enter_context(tc.tile_pool(name="catg", bufs=3))
        kg_p = ctx.enter_context(tc.tile_pool(name="kg", bufs=3))
        ctr_p = ctx.enter_context(tc.tile_pool(name="ctr", bufs=3))
        sb_p = ctx.enter_context(tc.tile_pool(name="work", bufs=5))
        small_p = ctx.enter_context(tc.tile_pool(name="small", bufs=6))
        ps_p = ctx.enter_context(tc.tile_pool(name="ps", bufs=1, space="PSUM"))

        for _rep in range(reps):
          for pr in range(pairs):
            # ---- gathers (pair granularity) ----------------------------
            cT3 = ctr_p.tile([128, 2, 256], bf16, tag="ctr")
            nc.gpsimd.dma_gather(
                cT3[:], catt.ap().bitcast(bf16)[:, 0:D],
                idxC[:, pr * 16:(pr + 1) * 16],
                256, 256, D, elem_step=2 * CATF, transpose=True)

            catg_f = catg_p.tile([128, 2 * K, 64], f32, tag="catg")
            nc.gpsimd.dma_gather(
                catg_f[:], catt.ap()[:, 192:256],
                idxN[:, pr * 256:(pr + 1) * 256],
                4096, 4096, 64, elem_step=CATF, single_packet=False)
            catg = catg_f[:].bitcast(fp8)          # [128, 32, 256]

            kg_f = kg_p.tile([128, 2 * K, 64], f32, tag="kg")
            nc.gpsimd.dma_gather(
                kg_f[:], catt.ap()[:, 128:192],
                idxK[:, pr * 256:(pr + 1) * 256],
                4096, 4096, 64, elem_step=CATF, single_packet=False)
            kgb = kg_f[:].bitcast(bf16)            # [128, 32, 128]

            for i in range(2):
                t = 2 * pr + i
                cstat0 = cT3[:, 0, i * 128:(i + 1) * 128]
                cstat1 = cT3[:, 1, i * 128:(i + 1) * 128]

                # ---- q = center @ Wq.T (SCALE folded in Wq);
                #      gate matmuls share the PSUM bank (col-disjoint) ---
                qg_ps = ps_p.tile([128, A + D], f32, tag="qg_ps",
                                  name="qg_ps")
                q_ps = qg_ps[:, 0:A]
                gate_ps = qg_ps[:, A:A + D]
                nc.tensor.matmul(q_ps, cstat0, wq[:, 0:A],
                                 start=True, stop=False)
                nc.tensor.matmul(q_ps, cstat1, wq[:, A:2 * A],
                                 start=False, stop=True)
                q_sb = small_p.tile([128, A], bf16, tag="q")
                nc.scalar.copy(q_sb[:], q_ps)

                # ---- gate: tanh form of sigmoid ------------------------
                nc.tensor.matmul(gate_ps, cstat0, wg[:, 0:D],
                                 start=True, stop=False)
                nc.tensor.matmul(gate_ps, cstat1, wg[:, D:2 * D],
                                 start=False, stop=False)
                nc.tensor.matmul(gate_ps, ones1, bg_row,
                                 start=False, stop=True)
                thp1 = sb_p.tile([128, D], bf16, tag="thp1")
                nc.scalar.activation(thp1[:], gate_ps, FN.Tanh, scale=0.5)

                # ---- logits: batched dot over A=64 ---------------------
                prod = sb_p.tile([128, K, A], bf16, tag="prod")
                nc.vector.tensor_tensor(
                    prod[:], kgb[:, K * i:K * (i + 1), 0:A],
                    q_sb[:, None, :].broadcast_to([128, K, A]), op=OP.mult)
                logits = small_p.tile([128, K], f32, tag="logits")
                nc.vector.tensor_reduce(logits[:], prod[:],
                                        axis=mybir.AxisListType.X, op=OP.add)

                # ---- softmax: e = exp(logits + logw), logw from host ---
                biased = small_p.tile([128, K], f32, tag="biased")
                nc.vector.tensor_tensor(biased[:], logits[:],
                                        logw[:, t * 16:(t + 1) * 16],
                                        op=OP.add)
                exps = small_p.tile([128, K], bf16, tag="exps")
                sums = small_p.tile([128, 1], f32, tag="sums")
                nc.scalar.activation(exps[:], biased[:], FN.Exp,
                                     accum_out=sums[:])
                recip = small_p.tile([128, 1], f32, tag="recip")
                nc.vector.reciprocal(recip[:], sums[:])
                e_n = small_p.tile([128, K], bf16, tag="e_n")
                nc.vector.tensor_scalar(e_n[:], exps[:], recip[:, 0:1], None,
                                        op0=OP.mult)

                # ---- block-diag attn stationaries ----------------------
                eT_ps = ps_p.tile([128, 128], bf16, tag="eT_ps", name="eT_ps")
                nc.tensor.transpose(eT_ps[0:K, :], e_n[:], idb)
                eT_sb = small_p.tile([K, 128], bf16, tag="eT_sb")
                nc.scalar.copy(eT_sb[:], eT_ps[0:K, :])
                erep_ps = ps_p.tile([128, 512], f32, tag="erep_ps",
                                    name="erep_ps")
                for m in range(4):
                    nc.tensor.matmul(erep_ps[:, m * 128:(m + 1) * 128],
                                     stat4[0:K, m * 128:(m + 1) * 128],
                                     eT_sb[:], start=True, stop=True)
                bigb = sb_p.tile([128, 512], bf16, tag="bigb")
                nc.vector.tensor_tensor(bigb[:], erep_ps[:], maskB4[:],
                                        op=OP.mult)

                # ---- context: 4 blocks x 4 accumulating matmuls --------
                ctx_ps = ps_p.tile([128, D], f32, tag="ctx_ps",
                                   name="ctx_ps", bufs=2)
                for tb in range(4):
                    for m in range(4):
                        nc.tensor.matmul(
                            ctx_ps[32 * tb:32 * (tb + 1), :],
                            bigb[:, m * 128 + 32 * tb:m * 128 + 32 * (tb + 1)],
                            catg[:, K * i + 4 * tb + m, :],
                            start=(m == 0), stop=(m == 3),
                            tile_position=(0, 32 * tb))

                # cg = (1+th)*ctx   (the sigmoid 1/2 lives in W2)
                ctxg = sb_p.tile([128, D], bf16, tag="ctxg")
                nc.vector.scalar_tensor_tensor(
                    out=ctxg[:], in0=thp1[:], scalar=1.0, in1=ctx_ps[:],
                    op0=OP.add, op1=OP.mult)
                # transpose gated context for the output projection
                cgT_ps = ps_p.tile([128, D], bf16, tag="cgT_ps",
                                   name="cgT_ps")
                nc.tensor.transpose(cgT_ps[:, 0:128], ctxg[:, 0:128], idb)
                nc.tensor.transpose(cgT_ps[:, 128:256], ctxg[:, 128:256], idb)
                cgT = sb_p.tile([128, 2, 128], bf16, tag="cgT")
                nc.scalar.copy(cgT[:, 0, :], cgT_ps[:, 0:128])
                nc.scalar.copy(cgT[:, 1, :], cgT_ps[:, 128:256])

                # ---- output projection + residual (folded) + bias ------
                x_ps = ps_p.tile([128, D], f32, tag="x_ps", name="x_ps",
                                 bufs=2)
                nc.tensor.matmul(x_ps[:], cstat0, w1[:, 0:D],
                                 start=True, stop=False)
                nc.tensor.matmul(x_ps[:], cstat1, w1[:, D:2 * D],
                                 start=False, stop=False)
                nc.tensor.matmul(x_ps[:], cgT[:, 0, :], w2[:, 0:D],
                                 start=False, stop=False)
                nc.tensor.matmul(x_ps[:], cgT[:, 1, :], w2[:, D:2 * D],
                                 start=False, stop=False)
                nc.tensor.matmul(x_ps[:], ones1, bo_row,
                                 start=False, stop=True)

                # ---- layernorm (rsqrt via recip + quadratic seed + NR) -
                bnst = small_p.tile([128, 6], f32, tag="bnst")
                nc.vector.bn_stats(bnst[:], x_ps[:])
                bnag = small_p.tile([128, 2], f32, tag="bnag")
                nc.vector.bn_aggr(bnag[:], bnst[:])
                var = bnag[:, 1:2]
                r_ = small_p.tile([128, 1], f32, tag="r_")
                nc.vector.reciprocal(r_[:], var)
                f1 = small_p.tile([128, 1], f32, tag="f1")
                nc.vector.tensor_scalar(f1[:], r_[:], RC, RB,
                                        op0=OP.mult, op1=OP.add)
                y0 = small_p.tile([128, 1], f32, tag="y0")
                nc.vector.scalar_tensor_tensor(
                    out=y0[:], in0=r_[:], scalar=RA, in1=f1[:],
                    op0=OP.add, op1=OP.mult)
                t2 = small_p.tile([128, 1], f32, tag="t2")
                nc.vector.tensor_tensor(t2[:], y0[:], y0[:], op=OP.mult)
                u = small_p.tile([128, 1], f32, tag="u")
                nc.vector.scalar_tensor_tensor(
                    out=u[:], in0=t2[:], scalar=-0.5, in1=var,
                    op0=OP.mult, op1=OP.mult)
                w_ = small_p.tile([128, 1], f32, tag="w_")
                nc.vector.tensor_scalar(w_[:], u[:], 1.5, None, op0=OP.add)
                rs = small_p.tile([128, 1], f32, tag="rs")
                nc.vector.tensor_tensor(rs[:], y0[:], w_[:], op=OP.mult)
                nmu = small_p.tile([128, 1], f32, tag="nmu")
                nc.vector.scalar_tensor_tensor(
                    out=nmu[:], in0=bnag[:, 0:1], scalar=-1.0, in1=rs[:],
                    op0=OP.mult, op1=OP.mult)
                xn = sb_p.tile([128, D], bf16, tag="xn")
                nc.scalar.activation(xn[:], x_ps[:], FN.Identity,
                                     bias=nmu[:], scale=rs[:])
                nc.sync.dma_start(out_d.ap()[t * 128:(t + 1) * 128, :], xn[:])

    nc.compile()
    return nc


# ---------------------------------------------------------------------------
# host-side input marshalling
# ---------------------------------------------------------------------------

def prep_core_inputs(all_embs, center_idx, nb_idx, nb_weights,
                     Wq, Wk, Wg, bg, Wo, bo, tiles=BC // 128):
    """Returns (shared_inputs, per_core_list) of numpy arrays."""
    import ml_dtypes
    bf = ml_dtypes.bfloat16
    f8 = ml_dtypes.float8_e4m3
    bc = tiles * 128
    pairs = tiles // 2
    ncores = B // BC if bc == BC else 1
    lay, CB = _blob_layout(tiles)

    E = np.ascontiguousarray(all_embs.astype(np.float32))
    kall = E @ (SCALE * Wk.astype(np.float32)).T                  # [N, A]
    row = np.zeros((N, 4 * CATF), np.uint8)
    row[:, 0:512] = np.ascontiguousarray(E.astype(bf)).view(np.uint8)
    row[:, 512:640] = np.ascontiguousarray(kall.astype(bf)).view(np.uint8)
    row[:, 768:1024] = np.ascontiguousarray(E.astype(f8)).view(np.uint8)
    catt = row.view(np.float32)

    WqT = np.ascontiguousarray(Wq.T.astype(np.float32))           # [D, A]
    WgT = np.ascontiguousarray(Wg.T.astype(np.float32))           # [D, D]
    W1 = Wo[:, :D].astype(np.float32) + np.eye(D, dtype=np.float32)
    W1T = np.ascontiguousarray(W1.T)
    W2T = np.ascontiguousarray(0.5 * Wo[:, D:].astype(np.float32).T)

    def chunk2(m):  # [D, X] -> [128, 2*X] bf16 (chunk-major)
        r = m.reshape(2, 128, -1).transpose(1, 0, 2).reshape(128, -1)
        return np.ascontiguousarray(r.astype(bf))

    # stat4[c, m*128+p] = (c == 4m + p%4), rows 0:16
    p = np.arange(128)
    stat4 = np.zeros((128, 512), bf)
    for m in range(4):
        stat4[4 * m + p % 4, m * 128 + p] = bf(1.0)
    # maskB[p, b] = (p//4 == b%32), tiled x4
    mb = (p[:, None] // 4 == p[None, :] % 32).astype(np.float32)
    maskB4 = np.ascontiguousarray(np.tile(mb, (1, 4)).astype(bf))

    def as_f32(a):
        return np.ascontiguousarray(a).view(np.float32)

    weights_blob = {
        "wq": as_f32(chunk2(WqT)),
        "wg": as_f32(chunk2(WgT)),
        "w1": as_f32(chunk2(W1T)),
        "w2": as_f32(chunk2(W2T)),
        "bgbo": as_f32(np.ascontiguousarray(np.broadcast_to(
            np.concatenate([bg, bo]).astype(bf)[None, :], (128, 2 * D)))),
        "identb": as_f32(np.eye(128, dtype=np.float32).astype(bf)),
        "stat4": as_f32(stat4),
        "maskB4": as_f32(maskB4),
        "ones": as_f32(np.ones((128, 128), bf)),
    }

    def wrap16(flat):
        """flat [n] in gather-position order -> [128, n/16] i16 wrapped in 16
        partitions and replicated x8."""
        w = flat.reshape(-1, 16).T.astype(np.int16)
        return np.ascontiguousarray(np.tile(w, (8, 1)))

    per_core = []
    for c in range(ncores):
        rows = slice(c * bc, (c + 1) * bc)
        nb = nb_idx[rows].astype(np.int64).reshape(pairs, 2, 128, K)
        # b-block order: position = 512t + 128m + 4u + jj for b=32t+u, j=4m+jj
        nbb = nb.reshape(pairs, 2, 4, 32, 4, 4).transpose(0, 1, 2, 4, 3, 5)
        nmat = np.concatenate(
            [wrap16(nbb[pq].reshape(-1)) for pq in range(pairs)], axis=1)
        # j-major order per tile: position = j*128 + b
        kmat = np.concatenate(
            [wrap16(np.concatenate(
                [nb[pq, ti].T.reshape(-1) for ti in range(2)]))
             for pq in range(pairs)], axis=1)

        ct = center_idx[rows].astype(np.int64)        # [bc]
        cmat = np.concatenate(
            [wrap16(ct[pq * 256:(pq + 1) * 256]) for pq in range(pairs)],
            axis=1)

        w = np.log(np.clip(nb_weights[rows].astype(np.float32), 1e-6,
                           None)).reshape(tiles, 128, K)
        wf = np.ascontiguousarray(
            w.transpose(1, 0, 2).reshape(128, tiles * K))

        blob = np.zeros((128, CB), np.float32)
        def put(name, arr):
            a, b = lay[name]
            assert arr.shape == (128, b - a), (name, arr.shape, b - a)
            blob[:, a:b] = arr
        put("nidx", nmat.view(np.float32))
        put("kidx", kmat.view(np.float32))
        put("cidx", cmat.view(np.float32))
        put("logw", wf)
        for nm, arr in weights_blob.items():
            put(nm, arr)
        per_core.append(dict(blob=blob))
    return dict(catt=catt), per_core


_CACHE = {}


def kernel(all_embs, center_idx, nb_idx, nb_weights, Wq, Wk, Wg, bg, Wo, bo,
           gamma, beta):
    from concourse.bass_utils import run_bass_kernel_spmd

    key = "full"
    if key not in _CACHE:
        _CACHE[key] = build_program()
    nc = _CACHE[key]

    shared, per_core = prep_core_inputs(
        np.asarray(all_embs), np.asarray(center_idx), np.asarray(nb_idx),
        np.asarray(nb_weights), np.asarray(Wq), np.asarray(Wk),
        np.asarray(Wg), np.asarray(bg), np.asarray(Wo), np.asarray(bo))

    in_maps = [{**shared, **pc} for pc in per_core]
    res = run_bass_kernel_spmd(nc, in_maps, list(range(NCORES)),
                               trace=bool(int(os.environ.get("KTRACE", "0"))))
    out = np.concatenate([res.results[c]["out"] for c in range(NCORES)],
                         axis=0).astype(np.float32)
    g = np.asarray(gamma, np.float32)
    bt = np.asarray(beta, np.float32)
    if not (np.all(g == 1.0) and np.all(bt == 0.0)):
        out = out * g[None, :] + bt[None, :]
    kernel.last_results = res
    return out
